# revision 32
# baseline (speedup 1.0000x reference)
"""Additive attention (B=4, Q=256, KV=1024, H=128, VS=256) on 8 Trainium2 cores.

Sharding: each core processes 32 query rows of every batch (4 groups of 32
row-slots).  Per batch, only a KV prefix of width ~valid_len (padded to even)
is computed; masked columns beyond it contribute exactly 0 to the softmax, so
skipping them is exact.  No collectives.  The program is specialized per
valid_lens configuration at call time and cached.

Per-core dataflow (ACT tanh is the hard floor: 1 elem/cycle/lane,
dtype-independent, ScalarE-only):
  PE  : k/q projections in fp32 (accuracy-critical, pre-tanh)
  DVE : feats[h, kv] = fp16(kp16[h, kv] + qp[h, s])  (tensor_scalar add,
        fp16 4x mode; kp stored fp16, qp scalar fp32)
  ACT : tanh in place over 8-row batches (the throughput floor, ~50us)
  PE  : single-pass fp16 one-hot matmuls accumulate score rows into a
        per-group PSUM tile (wv window with the weight at column 127-s);
        each group's mask row is written first via one K=1 matmul
  DVE/ACT/PE: per-group masked softmax (reduce_max / exp-with-accum-out row
        sums -> fp32 probs), PE band transposes into one PSUM strip tile,
        one fp16 cast copy, attn @ V in fp16 32-column bands; group i's
        softmax+attnV hide under group i+1's tanh stream.  One reciprocal +
        scale at the very end.
Queue discipline: Wk arrives in its own first DMA so the k0 projection can
start immediately; kT loads are interleaved with group 0's adds; softmax of
group i is emitted after sub-batch 0 of group i+1's tanh so no engine queue
blocks on a cross-engine round trip; V-tile DMAs trail kT on the sync queue.
"""
import math
import os
import sys

import numpy as np

for _p in ("/opt/trn_rl_repo", "/root/.axon_site/_ro/trn_rl_repo"):
    if os.path.isdir(_p):
        if _p not in sys.path:
            sys.path.insert(0, _p)
        break

B, Q, KV, QS, KS, H, VS = 4, 256, 1024, 128, 128, 128, 256
P = 128
N_CORES = 8
GROUP_ROWS = 32          # rows per (core, batch)
SUB = 8                  # rows per tanh batch
MASK_VAL = -30000.0      # large-negative that still fits fp16

PROFILE = False          # set by test.py; enables NTFF tracing
LO_PASS = True           # kept for test.py compat (unused in v2)
LAST_RESULTS = None
SIMULATE = False         # set by test.py; run CoreSim instead of hardware
LAST_EXEC_NS = None

_prog_cache = {}


def _build_program(cfg):
    """cfg: (Ws, l0flags): per-group computed KV widths in processing order
    and per-group valid_len==0 flags.  Returns nc."""
    Ws, l0flags = cfg
    import contextlib

    import concourse.bacc as bacc
    import concourse.mybir as mybir
    import concourse.tile as tile
    from concourse.tile_rust import add_dep_helper

    f32 = mybir.dt.float32
    f16 = mybir.dt.float16
    W = list(Ws)
    Wmax = max(W)
    SW = sum(W)
    offs = [sum(W[:i]) for i in range(B)]          # kp_sb column offsets
    nstrips = [(w + P - 1) // P for w in W]
    NSB = GROUP_ROWS // SUB
    nc = bacc.Bacc("TRN2", target_bir_lowering=False, debug=False,
                   enable_asserts=True, num_devices=N_CORES)

    wk_d = nc.dram_tensor("wk", [P, P], f16, kind="ExternalInput").ap()
    blobq_d = nc.dram_tensor("blobq", [P, 2 * P], f32,
                             kind="ExternalInput").ap()
    kT_d = nc.dram_tensor("kT", [P, SW], f16, kind="ExternalInput").ap()
    V_d = nc.dram_tensor("V", [B, KV, VS], f16, kind="ExternalInput").ap()
    wvd_d = nc.dram_tensor("wvd", [P, 2 * P - 1], f16, kind="ExternalInput").ap()
    ind_d = nc.dram_tensor("ind", [2, B * P], f16, kind="ExternalInput").ap()
    mask_d = nc.dram_tensor("mask", [2, B * Wmax], f16, kind="ExternalInput").ap()
    bident_d = nc.dram_tensor("bident", [P, GROUP_ROWS], f16,
                              kind="ExternalInput").ap()
    out_d = nc.dram_tensor("out", [P, VS], f32, kind="ExternalOutput").ap()

    with tile.TileContext(nc) as tc, contextlib.ExitStack() as ctx:
        const = ctx.enter_context(tc.tile_pool(name="const", bufs=1))
        ktp = ctx.enter_context(tc.tile_pool(name="ktp", bufs=4))
        featsp = ctx.enter_context(tc.tile_pool(name="featsp", bufs=5))
        probsp = ctx.enter_context(tc.tile_pool(name="probsp", bufs=2))
        small = ctx.enter_context(tc.tile_pool(name="small", bufs=3))
        scp = ctx.enter_context(tc.tile_pool(name="scp", bufs=2, space="PSUM"))
        pmix = ctx.enter_context(tc.tile_pool(name="pmix", bufs=3, space="PSUM"))
        outp = ctx.enter_context(tc.tile_pool(name="outp", bufs=1, space="PSUM"))

        # ---- ACT table warm-up: load the exp/tanh spline set while the
        # first DMAs are still in flight ----
        warm = const.tile([1, 2], f16)
        nc.gpsimd.memset(warm[:], 0.0)
        nc.scalar.activation(warm[:], warm[:],
                             mybir.ActivationFunctionType.Tanh)

        # ---- constant loads: wk first (k0 projection gates the pipeline),
        # then kT0, then Wq + the first 32 qT columns (all group 0 needs),
        # then the rest; V tiles trail kT on the Sync queue; small leftovers
        # on the idle GpSimd issue queue ----
        wk_sb_t = const.tile([P, P], f16)
        nc.sync.dma_start(wk_sb_t[:], wk_d[:])
        wk_sb = wk_sb_t[:]
        blobq = const.tile([P, 2 * P], f32)
        wq_sb = blobq[:, 0:P]
        qt_sb = blobq[:, P:2 * P]
        wvd_t = const.tile([P, 2 * P - 1], f16)
        nc.gpsimd.dma_start(wvd_t[:], wvd_d[:])
        ind_sb = const.tile([2, B * P], f16)
        nc.gpsimd.dma_start(ind_sb[:], ind_d[:])
        mask_sb = const.tile([2, B * Wmax], f16)
        nc.gpsimd.dma_start(mask_sb[:], mask_d[:])
        bident = const.tile([P, GROUP_ROWS], f16)
        nc.gpsimd.dma_start(bident[:], bident_d[:])

        kp_sb = const.tile([P, SW], f16)
        out_ps = outp.tile([P, VS], f32, name="out_ps")
        out_sb = const.tile([P, VS], f32)
        rinv = small.tile([P, 1], f32, bufs=1, tag="rinv")
        vts = {}
        scores = [None] * B
        last_kp_copy = [None]

        kp_pss = {}

        def emit_load_mm(i):
            """kT chunk DMAs + fp32 projections for group i (PE/sync
            side); chunked so the first projection starts as soon as the
            first 512 columns land."""
            w = W[i]
            for c0 in range(0, w, 512):
                n = min(512, w - c0)
                kt_t = ktp.tile([P, 512], f16, tag="kt", name=f"kt_{i}_{c0}")
                nc.sync.dma_start(kt_t[:, :n], kT_d[:, offs[i] + c0:
                                                   offs[i] + c0 + n])
                kp_ps = pmix.tile([P, 512], f32, tag="mix",
                                  name=f"kp_ps_{i}_{c0}")
                nc.tensor.matmul(kp_ps[:, :n], wk_sb, kt_t[:, :n],
                                 start=True, stop=True)
                kp_pss[(i, c0)] = kp_ps

        def emit_load_cast(i):
            """fp16 kp copies for group i (DVE side, emitted late so the
            casts never block earlier adds)."""
            w = W[i]
            for c0 in range(0, w, 512):
                n = min(512, w - c0)
                last_kp_copy[0] = nc.vector.tensor_copy(
                    kp_sb[:, offs[i] + c0: offs[i] + c0 + n],
                    kp_pss[(i, c0)][:, :n])

        def emit_mask(i):
            # K=2 rank-2 init: rows in the band get the valid-len mask, rows
            # outside it get MASK_VAL so they exp to exactly 0 later (the
            # probs->pt reduction matmul sums over all four bands).
            w = W[i]
            sc = scp.tile([P, w], f32, tag="sc", name=f"scores_{i}")
            scores[i] = sc
            for c0 in range(0, w, 512):
                c1 = min(c0 + 512, w)
                nc.tensor.matmul(
                    sc[:, c0:c1],
                    ind_sb[0:2, i * P:(i + 1) * P],
                    mask_sb[0:2, i * Wmax + c0: i * Wmax + c1],
                    start=True, stop=l0flags[i] and c1 == w,
                    skip_group_check=True)

        def emit_scores(i, ranges):
            """adds + tanh + one-hot score matmuls for group i over the
            given (row0, row1) ranges."""
            w = W[i]
            sc = scores[i]
            for r0, r1 in ranges:
                nr = r1 - r0
                feats = featsp.tile([P, nr * w], f16, tag="feats",
                                    name=f"feats_{i}_{r0}")
                for j in range(nr):
                    s = GROUP_ROWS * i + r0 + j
                    nc.vector.tensor_scalar_add(
                        feats[:, j * w:(j + 1) * w],
                        kp_sb[:, offs[i]: offs[i] + w],
                        qp_sb[:, s: s + 1])
                nc.scalar.activation(feats[:], feats[:],
                                     mybir.ActivationFunctionType.Tanh)
                for j in range(nr):
                    s = GROUP_ROWS * i + r0 + j
                    last_row = r0 + j == GROUP_ROWS - 1
                    for c0 in range(0, w, 512):
                        c1 = min(c0 + 512, w)
                        nc.tensor.matmul(
                            sc[:, c0:c1],
                            wvd_t[:, P - 1 - s: 2 * P - 1 - s],
                            feats[:, j * w + c0: j * w + c1],
                            start=False,
                            stop=last_row and c1 == w,
                            skip_group_check=True)

        def emit_vdma(i):
            for c in range(nstrips[i]):
                cw = min(P, W[i] - c * P)
                vts[(i, c)] = const.tile([P, VS], f16, name=f"v_{i}_{c}")
                vdma = nc.sync.dma_start(vts[(i, c)][:cw, :],
                                         V_d[i, c * P: c * P + cw, :])
                if last_kp_copy[0] is not None:
                    add_dep_helper(vdma.ins, last_kp_copy[0].ins,
                                   reason="V after kp: kT wins head HBM bw")

        nrms = [None] * B

        def emit_rmax(i):
            # per-group -max; 0 outside the band so those rows (scores
            # MASK_VAL) exp to exactly 0.  Split from the rest of the
            # softmax so the last group's reduce_max is not queued on DVE
            # behind the previous group's exp-gated ops.
            sc = scores[i]
            band = slice(GROUP_ROWS * i, GROUP_ROWS * (i + 1))
            nrm = small.tile([P, 1], f32, bufs=2, tag="nrm",
                             name=f"nrm_{i}")
            nrms[i] = nrm
            nc.vector.memset(nrm[:], 0.0)
            nc.vector.reduce_max(nrm[band, :], sc[band, :],
                                 axis=mybir.AxisListType.X, negate=True)

        def emit_softmax_attnv(i):
            w = W[i]
            n = nstrips[i]
            sc = scores[i]
            nrm = nrms[i]
            band = slice(GROUP_ROWS * i, GROUP_ROWS * (i + 1))
            wpad = n * P
            probs = probsp.tile([P, wpad], f16, tag="probs",
                                name=f"probs_{i}")
            rs = small.tile([P, 1], f32, bufs=4, tag="rs", name=f"rs_{i}")
            if wpad > w:
                nc.vector.memset(probs[:, w:], 0.0)
            nc.scalar.activation(probs[:, :w], sc[:],
                                 mybir.ActivationFunctionType.Exp,
                                 bias=nrm[:, 0:1], scale=1.0,
                                 accum_out=rs[:, 0:1])
            nc.vector.reciprocal(rinv[band, :], rs[band, :])
            # "transpose" probs via probs.T @ band-identity: full-height
            # stationary (base partition 0 -- quadrant-3-safe); non-band
            # rows are exactly 0 so the cross-band sum picks out the band
            pt_ps = pmix.tile([P, GROUP_ROWS * n], f32, tag="mix",
                              name=f"pt_ps_{i}")
            for c in range(n):
                nc.tensor.matmul(pt_ps[:, GROUP_ROWS * c:
                                       GROUP_ROWS * (c + 1)],
                                 probs[:, c * P:(c + 1) * P],
                                 bident[:, :],
                                 start=True, stop=True,
                                 skip_group_check=True)
            pt_sb = small.tile([P, GROUP_ROWS * n], f16, tag="pt",
                               name=f"pt_sb_{i}")
            nc.vector.tensor_copy(pt_sb[:], pt_ps[:])
            for c in range(n):
                cw = min(P, w - c * P)
                nc.tensor.matmul(
                    out_ps[band, :],
                    pt_sb[:cw, GROUP_ROWS * c: GROUP_ROWS * (c + 1)],
                    vts[(i, c)][:cw, :],
                    start=(c == 0), stop=(c == n - 1),
                    tile_position=(0, GROUP_ROWS * i),
                    skip_group_check=True)
            # scale + ship this band now: only the last group's output DMA
            # lands in the tail
            nc.vector.tensor_scalar_mul(out_sb[band, :], out_ps[band, :],
                                        rinv[band, 0:1])
            nc.sync.dma_start(out_d[GROUP_ROWS * i: GROUP_ROWS * (i + 1), :],
                              out_sb[band, :])

        # ---- head: kT0 on the sync queue; q-side DMAs go down the GpSimd
        # queue in parallel so neither serializes behind the other ----
        emit_load_mm(0)
        nc.gpsimd.dma_start(wq_sb, blobq_d[:, 0:P])
        nc.gpsimd.dma_start(qt_sb[:, 0:GROUP_ROWS],
                            blobq_d[:, P:P + GROUP_ROWS])
        nc.gpsimd.dma_start(qt_sb[:, GROUP_ROWS:],
                            blobq_d[:, P + GROUP_ROWS:2 * P])
        qp_ps = pmix.tile([P, P], f32, tag="mix", name="qp_ps")
        nc.tensor.matmul(qp_ps[:, 0:GROUP_ROWS], wq_sb,
                         qt_sb[:, 0:GROUP_ROWS], start=True, stop=True,
                         skip_group_check=True)
        qp_sb = const.tile([P, P], f32)
        nc.vector.tensor_copy(qp_sb[:, 0:GROUP_ROWS], qp_ps[:, 0:GROUP_ROWS])
        nc.tensor.matmul(qp_ps[:, GROUP_ROWS:], wq_sb,
                         qt_sb[:, GROUP_ROWS:], start=True, stop=True,
                         skip_group_check=True)
        nc.vector.tensor_copy(qp_sb[:, GROUP_ROWS:], qp_ps[:, GROUP_ROWS:])
        emit_load_cast(0)
        emit_mask(0)
        if not l0flags[0]:
            emit_scores(0, [(0, 4), (4, 8)])
            emit_load_mm(1)
            emit_scores(0, [(8, 16)])
            emit_load_mm(2)
            emit_load_mm(3)
            emit_scores(0, [(16, 32)])
        else:
            emit_load_mm(1)
            emit_load_mm(2)
            emit_load_mm(3)
        emit_load_cast(1)
        emit_vdma(0)

        # ---- main loop: group i's first tanh batch precedes group i-1's
        # softmax; the last group's reduce_max precedes the exp-gated DVE
        # work of groups B-2/B-1 so the tail chain starts immediately ----
        for i in range(1, B):
            emit_mask(i)
            emit_vdma(i)
            if i + 1 < B:
                emit_load_cast(i + 1)
            if not l0flags[i]:
                emit_scores(i, [(0, 16)])
                emit_rmax(i - 1)
                if i < B - 1:
                    emit_scores(i, [(16, 32)])
                    emit_softmax_attnv(i - 1)
                else:
                    emit_scores(i, [(16, 24), (24, 32)])
                    emit_rmax(i)
                    # last group's softmax first: its chain is the tail;
                    # group B-2's finishes in parallel
                    emit_softmax_attnv(i)
                    emit_softmax_attnv(i - 1)
            else:
                emit_rmax(i - 1)
                if i == B - 1:
                    emit_rmax(i)
                    emit_softmax_attnv(i)
                emit_softmax_attnv(i - 1)

    nc.compile()
    return nc


def _get_program(cfg):
    if cfg not in _prog_cache:
        _prog_cache[cfg] = _build_program(cfg)
    return _prog_cache[cfg]


def _width(L):
    # even-padded computed width; valid_len==0 means "uniform over all KV"
    if L <= 0:
        return KV
    L = min(L, KV)
    return min(KV, max(2, 2 * math.ceil(L / 2)))


def kernel(queries, keys, values, valid_lens, Wq, Wk, wv):
    global LAST_EXEC_NS
    queries = np.ascontiguousarray(np.asarray(queries), dtype=np.float32)
    keys = np.ascontiguousarray(np.asarray(keys), dtype=np.float32)
    values = np.ascontiguousarray(np.asarray(values), dtype=np.float32)
    Wq = np.ascontiguousarray(np.asarray(Wq), dtype=np.float32)
    Wk = np.ascontiguousarray(np.asarray(Wk), dtype=np.float32)
    wv = np.ascontiguousarray(np.asarray(wv), dtype=np.float32)
    vl = [int(x) for x in np.asarray(valid_lens)]

    W_b = [_width(L) for L in vl]
    # widest group first: its long tanh stream gives the DVE adds of every
    # later group enough runway; smallest group last for a short tail
    gorder = sorted(range(B), key=lambda b: (-W_b[b], b))
    Ws = tuple(W_b[b] for b in gorder)
    l0flags = tuple(vl[b] == 0 for b in gorder)
    Wmax = max(Ws)

    nc = _get_program((Ws, l0flags))

    kT = np.concatenate(
        [keys[gorder[i]][:Ws[i]].T for i in range(B)], axis=1)
    kT = np.ascontiguousarray(kT.astype(np.float16))     # [128, SW]
    Vm = np.ascontiguousarray(
        np.stack([values[gorder[i]] for i in range(B)]).astype(np.float16))
    # row 0: band indicator x per-group valid mask; row 1: outside-band
    # indicator x MASK_VAL (so non-band score rows exp to exactly 0)
    ind = np.zeros((2, B * P), np.float16)
    for i in range(B):
        ind[0, i * P + GROUP_ROWS * i: i * P + GROUP_ROWS * (i + 1)] = 1.0
        ind[1, i * P: (i + 1) * P] = 1.0
        ind[1, i * P + GROUP_ROWS * i: i * P + GROUP_ROWS * (i + 1)] = 0.0
    mask = np.zeros((2, B * Wmax), np.float16)
    mask[1, :] = MASK_VAL
    for i in range(B):
        L = vl[gorder[i]]
        if L > 0:
            mask[0, i * Wmax + min(L, Ws[i]): i * Wmax + Ws[i]] = MASK_VAL
    wvd = np.zeros((P, 2 * P - 1), np.float16)
    wvd[:, P - 1] = wv.astype(np.float16)
    bident = np.ascontiguousarray(
        np.tile(np.eye(GROUP_ROWS, dtype=np.float16), (B, 1)))

    blobq = np.zeros((P, 2 * P), np.float32)
    blobq[:, 0:P] = Wq
    shared = {"wk": np.ascontiguousarray(Wk.astype(np.float16)), "kT": kT, "V": Vm, "ind": ind,
              "mask": mask, "wvd": wvd, "bident": bident}
    in_maps = []
    for c in range(N_CORES):
        qT = np.concatenate(
            [queries[gorder[i], c * GROUP_ROWS:(c + 1) * GROUP_ROWS, :].T
             for i in range(B)], axis=1)
        bl = blobq.copy()
        bl[:, P:2 * P] = qT
        m = dict(shared)
        m["blobq"] = bl
        in_maps.append(m)

    if SIMULATE:
        from concourse.bass_interp import CoreSim
        outs = []
        for c in range(N_CORES):
            sim = CoreSim(nc, trace=False)
            for name, v in in_maps[c].items():
                sim.tensor(name)[:] = v
            sim.simulate(check_with_hw=False)
            outs.append(sim.tensor("out").copy())
    else:
        from concourse import bass_utils
        kw = {}
        if PROFILE:
            kw = {"trace": True}
        res = bass_utils.run_bass_kernel_spmd(nc, in_maps, list(range(N_CORES)),
                                              **kw)
        if PROFILE:
            LAST_EXEC_NS = res.exec_time_ns
            global LAST_RESULTS
            LAST_RESULTS = res
        outs = [res.results[c]["out"] for c in range(N_CORES)]

    out = np.zeros((B, Q, VS), np.float32)
    for c in range(N_CORES):
        for i in range(B):
            out[gorder[i], c * GROUP_ROWS:(c + 1) * GROUP_ROWS, :] = \
                outs[c][GROUP_ROWS * i: GROUP_ROWS * (i + 1), :]
    return out


# revision 33
# speedup vs baseline: 1.0222x; 1.0222x over previous
"""Additive attention (B=4, Q=256, KV=1024, H=128, VS=256) on 8 Trainium2 cores.

Sharding: each core processes 32 query rows of every batch (4 groups of 32
row-slots).  Per batch, only a KV prefix of width ~valid_len (padded to even)
is computed; masked columns beyond it contribute exactly 0 to the softmax, so
skipping them is exact.  No collectives.  The program is specialized per
valid_lens configuration at call time and cached.

Per-core dataflow (ACT tanh is the hard floor: 1 elem/cycle/lane,
dtype-independent, ScalarE-only):
  PE  : k/q projections in fp32 (accuracy-critical, pre-tanh)
  DVE : feats[h, kv] = fp16(kp16[h, kv] + qp[h, s])  (tensor_scalar add,
        fp16 4x mode; kp stored fp16, qp scalar fp32)
  ACT : tanh in place over 8-row batches (the throughput floor, ~50us)
  PE  : single-pass fp16 one-hot matmuls accumulate score rows into a
        per-group PSUM tile (wv window with the weight at column 127-s);
        each group's mask row is written first via one K=1 matmul
  DVE/ACT/PE: per-group masked softmax (reduce_max / exp-with-accum-out row
        sums -> fp32 probs), PE band transposes into one PSUM strip tile,
        one fp16 cast copy, attn @ V in fp16 32-column bands; group i's
        softmax+attnV hide under group i+1's tanh stream.  One reciprocal +
        scale at the very end.
Queue discipline: Wk arrives in its own first DMA so the k0 projection can
start immediately; kT loads are interleaved with group 0's adds; softmax of
group i is emitted after sub-batch 0 of group i+1's tanh so no engine queue
blocks on a cross-engine round trip; V-tile DMAs trail kT on the sync queue.
"""
import math
import os
import sys

import numpy as np

for _p in ("/opt/trn_rl_repo", "/root/.axon_site/_ro/trn_rl_repo"):
    if os.path.isdir(_p):
        if _p not in sys.path:
            sys.path.insert(0, _p)
        break

B, Q, KV, QS, KS, H, VS = 4, 256, 1024, 128, 128, 128, 256
P = 128
N_CORES = 8
GROUP_ROWS = 32          # rows per (core, batch)
SUB = 8                  # rows per tanh batch
MASK_VAL = -30000.0      # large-negative that still fits fp16

PROFILE = False          # set by test.py; enables NTFF tracing
LO_PASS = True           # kept for test.py compat (unused in v2)
LAST_RESULTS = None
SIMULATE = False         # set by test.py; run CoreSim instead of hardware
LAST_EXEC_NS = None

_prog_cache = {}


def _build_program(cfg):
    """cfg: (Ws, l0flags): per-group computed KV widths in processing order
    and per-group valid_len==0 flags.  Returns nc."""
    Ws, l0flags = cfg
    import contextlib

    import concourse.bacc as bacc
    import concourse.mybir as mybir
    import concourse.tile as tile
    from concourse.tile_rust import add_dep_helper

    f32 = mybir.dt.float32
    f16 = mybir.dt.float16
    W = list(Ws)
    Wmax = max(W)
    SW = sum(W)
    offs = [sum(W[:i]) for i in range(B)]          # kp_sb column offsets
    nstrips = [(w + P - 1) // P for w in W]
    NSB = GROUP_ROWS // SUB
    nc = bacc.Bacc("TRN2", target_bir_lowering=False, debug=False,
                   enable_asserts=True, num_devices=N_CORES)

    wk_d = nc.dram_tensor("wk", [P, P], f16, kind="ExternalInput").ap()
    blobq_d = nc.dram_tensor("blobq", [P, 2 * P], f32,
                             kind="ExternalInput").ap()
    kT_d = nc.dram_tensor("kT", [P, SW], f16, kind="ExternalInput").ap()
    V_d = nc.dram_tensor("V", [B, KV, VS], f16, kind="ExternalInput").ap()
    wvd_d = nc.dram_tensor("wvd", [P, 2 * P - 1], f16, kind="ExternalInput").ap()
    ind_d = nc.dram_tensor("ind", [2, B * P], f16, kind="ExternalInput").ap()
    mask_d = nc.dram_tensor("mask", [2, B * Wmax], f16, kind="ExternalInput").ap()
    bident_d = nc.dram_tensor("bident", [P, GROUP_ROWS], f16,
                              kind="ExternalInput").ap()
    out_d = nc.dram_tensor("out", [P, VS], f32, kind="ExternalOutput").ap()

    with tile.TileContext(nc) as tc, contextlib.ExitStack() as ctx:
        const = ctx.enter_context(tc.tile_pool(name="const", bufs=1))
        ktp = ctx.enter_context(tc.tile_pool(name="ktp", bufs=4))
        featsp = ctx.enter_context(tc.tile_pool(name="featsp", bufs=5))
        probsp = ctx.enter_context(tc.tile_pool(name="probsp", bufs=2))
        small = ctx.enter_context(tc.tile_pool(name="small", bufs=3))
        scp = ctx.enter_context(tc.tile_pool(name="scp", bufs=2, space="PSUM"))
        pmix = ctx.enter_context(tc.tile_pool(name="pmix", bufs=3, space="PSUM"))
        outp = ctx.enter_context(tc.tile_pool(name="outp", bufs=1, space="PSUM"))

        # ---- ACT table warm-up: load the exp/tanh spline set while the
        # first DMAs are still in flight ----
        warm = const.tile([1, 2], f16)
        nc.gpsimd.memset(warm[:], 0.0)
        nc.scalar.activation(warm[:], warm[:],
                             mybir.ActivationFunctionType.Tanh)

        # ---- constant loads: wk first (k0 projection gates the pipeline),
        # then kT0, then Wq + the first 32 qT columns (all group 0 needs),
        # then the rest; V tiles trail kT on the Sync queue; small leftovers
        # on the idle GpSimd issue queue ----
        wk_sb_t = const.tile([P, P], f16)
        nc.sync.dma_start(wk_sb_t[:], wk_d[:])
        wk_sb = wk_sb_t[:]
        blobq = const.tile([P, 2 * P], f32)
        wq_sb = blobq[:, 0:P]
        qt_sb = blobq[:, P:2 * P]
        wvd_t = const.tile([P, 2 * P - 1], f16)
        nc.gpsimd.dma_start(wvd_t[:], wvd_d[:])
        ind_sb = const.tile([2, B * P], f16)
        nc.gpsimd.dma_start(ind_sb[:], ind_d[:])
        mask_sb = const.tile([2, B * Wmax], f16)
        nc.gpsimd.dma_start(mask_sb[:], mask_d[:])
        bident = const.tile([P, GROUP_ROWS], f16)
        nc.gpsimd.dma_start(bident[:], bident_d[:])

        kp_sb = const.tile([P, SW], f16)
        out_ps = outp.tile([P, VS], f32, name="out_ps")
        out_sb = const.tile([P, VS], f32)
        rinv = small.tile([P, 1], f32, bufs=1, tag="rinv")
        vts = {}
        scores = [None] * B
        last_kp_copy = [None]

        kp_pss = {}

        def emit_load_mm(i):
            """kT chunk DMAs + fp32 projections for group i (PE/sync
            side); chunked so the first projection starts as soon as the
            first 512 columns land."""
            w = W[i]
            for c0 in range(0, w, 512):
                n = min(512, w - c0)
                kt_t = ktp.tile([P, 512], f16, tag="kt", name=f"kt_{i}_{c0}")
                nc.sync.dma_start(kt_t[:, :n], kT_d[:, offs[i] + c0:
                                                   offs[i] + c0 + n])
                kp_ps = pmix.tile([P, 512], f32, tag="mix",
                                  name=f"kp_ps_{i}_{c0}")
                nc.tensor.matmul(kp_ps[:, :n], wk_sb, kt_t[:, :n],
                                 start=True, stop=True)
                kp_pss[(i, c0)] = kp_ps

        def emit_load_cast(i):
            """fp16 kp copies for group i (DVE side, emitted late so the
            casts never block earlier adds)."""
            w = W[i]
            for c0 in range(0, w, 512):
                n = min(512, w - c0)
                last_kp_copy[0] = nc.vector.tensor_copy(
                    kp_sb[:, offs[i] + c0: offs[i] + c0 + n],
                    kp_pss[(i, c0)][:, :n])

        def emit_mask(i):
            # K=2 rank-2 init: rows in the band get the valid-len mask, rows
            # outside it get MASK_VAL so they exp to exactly 0 later (the
            # probs->pt reduction matmul sums over all four bands).
            w = W[i]
            sc = scp.tile([P, w], f32, tag="sc", name=f"scores_{i}")
            scores[i] = sc
            for c0 in range(0, w, 512):
                c1 = min(c0 + 512, w)
                nc.tensor.matmul(
                    sc[:, c0:c1],
                    ind_sb[0:2, i * P:(i + 1) * P],
                    mask_sb[0:2, i * Wmax + c0: i * Wmax + c1],
                    start=True, stop=l0flags[i] and c1 == w,
                    skip_group_check=True)

        def emit_scores(i, ranges):
            """adds + tanh + one-hot score matmuls for group i over the
            given (row0, row1) ranges."""
            w = W[i]
            sc = scores[i]
            for r0, r1 in ranges:
                nr = r1 - r0
                feats = featsp.tile([P, nr * w], f16, tag="feats",
                                    name=f"feats_{i}_{r0}")
                for j in range(nr):
                    s = GROUP_ROWS * i + r0 + j
                    nc.vector.tensor_scalar_add(
                        feats[:, j * w:(j + 1) * w],
                        kp_sb[:, offs[i]: offs[i] + w],
                        qp_sb[:, s: s + 1])
                nc.scalar.activation(feats[:], feats[:],
                                     mybir.ActivationFunctionType.Tanh)
                for j in range(nr):
                    s = GROUP_ROWS * i + r0 + j
                    last_row = r0 + j == GROUP_ROWS - 1
                    for c0 in range(0, w, 512):
                        c1 = min(c0 + 512, w)
                        nc.tensor.matmul(
                            sc[:, c0:c1],
                            wvd_t[:, P - 1 - s: 2 * P - 1 - s],
                            feats[:, j * w + c0: j * w + c1],
                            start=False,
                            stop=last_row and c1 == w,
                            skip_group_check=True)

        def emit_vdma(i):
            for c in range(nstrips[i]):
                cw = min(P, W[i] - c * P)
                vts[(i, c)] = const.tile([P, VS], f16, name=f"v_{i}_{c}")
                vdma = nc.sync.dma_start(vts[(i, c)][:cw, :],
                                         V_d[i, c * P: c * P + cw, :])
                if last_kp_copy[0] is not None:
                    add_dep_helper(vdma.ins, last_kp_copy[0].ins,
                                   reason="V after kp: kT wins head HBM bw")

        nrms = [None] * B

        def emit_rmax(i):
            # per-group -max; 0 outside the band so those rows (scores
            # MASK_VAL) exp to exactly 0.  Split from the rest of the
            # softmax so the last group's reduce_max is not queued on DVE
            # behind the previous group's exp-gated ops.
            sc = scores[i]
            band = slice(GROUP_ROWS * i, GROUP_ROWS * (i + 1))
            nrm = small.tile([P, 1], f32, bufs=2, tag="nrm",
                             name=f"nrm_{i}")
            nrms[i] = nrm
            nc.vector.memset(nrm[:], 0.0)
            nc.vector.reduce_max(nrm[band, :], sc[band, :],
                                 axis=mybir.AxisListType.X, negate=True)

        def emit_softmax_attnv(i):
            w = W[i]
            n = nstrips[i]
            sc = scores[i]
            nrm = nrms[i]
            band = slice(GROUP_ROWS * i, GROUP_ROWS * (i + 1))
            wpad = n * P
            probs = probsp.tile([P, wpad], f16, tag="probs",
                                name=f"probs_{i}")
            rs = small.tile([P, 1], f32, bufs=4, tag="rs", name=f"rs_{i}")
            if wpad > w:
                nc.vector.memset(probs[:, w:], 0.0)
            nc.scalar.activation(probs[:, :w], sc[:],
                                 mybir.ActivationFunctionType.Exp,
                                 bias=nrm[:, 0:1], scale=1.0,
                                 accum_out=rs[:, 0:1])
            nc.vector.reciprocal(rinv[band, :], rs[band, :])
            # "transpose" probs via probs.T @ band-identity: full-height
            # stationary (base partition 0 -- quadrant-3-safe); non-band
            # rows are exactly 0 so the cross-band sum picks out the band
            pt_ps = pmix.tile([P, GROUP_ROWS * n], f32, tag="mix",
                              name=f"pt_ps_{i}")
            for c in range(n):
                nc.tensor.matmul(pt_ps[:, GROUP_ROWS * c:
                                       GROUP_ROWS * (c + 1)],
                                 probs[:, c * P:(c + 1) * P],
                                 bident[:, :],
                                 start=True, stop=True,
                                 skip_group_check=True)
            pt_sb = small.tile([P, GROUP_ROWS * n], f16, tag="pt",
                               name=f"pt_sb_{i}")
            nc.vector.tensor_copy(pt_sb[:], pt_ps[:])
            for c in range(n):
                cw = min(P, w - c * P)
                nc.tensor.matmul(
                    out_ps[band, :],
                    pt_sb[:cw, GROUP_ROWS * c: GROUP_ROWS * (c + 1)],
                    vts[(i, c)][:cw, :],
                    start=(c == 0), stop=(c == n - 1),
                    tile_position=(0, GROUP_ROWS * i),
                    skip_group_check=True)
            # scale + ship this band now: only the last group's output DMA
            # lands in the tail
            nc.vector.tensor_scalar_mul(out_sb[band, :], out_ps[band, :],
                                        rinv[band, 0:1])
            nc.sync.dma_start(out_d[GROUP_ROWS * i: GROUP_ROWS * (i + 1), :],
                              out_sb[band, :])

        # ---- head: kT0 + group-0 q columns first on the sync queue (the
        # GpSimd queue's SWDGE descriptor generation is ~650ns per DMA,
        # too slow for the head-critical q data) ----
        emit_load_mm(0)
        nc.sync.dma_start(wq_sb, blobq_d[:, 0:P])
        nc.sync.dma_start(qt_sb[:, 0:GROUP_ROWS],
                          blobq_d[:, P:P + GROUP_ROWS])
        nc.sync.dma_start(qt_sb[:, GROUP_ROWS:],
                          blobq_d[:, P + GROUP_ROWS:2 * P])
        qp_ps = pmix.tile([P, P], f32, tag="mix", name="qp_ps")
        nc.tensor.matmul(qp_ps[:, 0:GROUP_ROWS], wq_sb,
                         qt_sb[:, 0:GROUP_ROWS], start=True, stop=True,
                         skip_group_check=True)
        qp_sb = const.tile([P, P], f32)
        nc.vector.tensor_copy(qp_sb[:, 0:GROUP_ROWS], qp_ps[:, 0:GROUP_ROWS])
        nc.tensor.matmul(qp_ps[:, GROUP_ROWS:], wq_sb,
                         qt_sb[:, GROUP_ROWS:], start=True, stop=True,
                         skip_group_check=True)
        nc.vector.tensor_copy(qp_sb[:, GROUP_ROWS:], qp_ps[:, GROUP_ROWS:])
        emit_load_cast(0)
        emit_mask(0)
        if not l0flags[0]:
            emit_scores(0, [(0, 4), (4, 8)])
            emit_load_mm(1)
            emit_scores(0, [(8, 16)])
            emit_load_mm(2)
            emit_load_mm(3)
            emit_scores(0, [(16, 32)])
        else:
            emit_load_mm(1)
            emit_load_mm(2)
            emit_load_mm(3)
        emit_load_cast(1)
        emit_vdma(0)

        # ---- main loop: group i's first tanh batch precedes group i-1's
        # softmax; the last group's reduce_max precedes the exp-gated DVE
        # work of groups B-2/B-1 so the tail chain starts immediately ----
        for i in range(1, B):
            emit_mask(i)
            emit_vdma(i)
            if i + 1 < B:
                emit_load_cast(i + 1)
            if not l0flags[i]:
                emit_scores(i, [(0, 16)])
                emit_rmax(i - 1)
                if i < B - 1:
                    emit_scores(i, [(16, 32)])
                    emit_softmax_attnv(i - 1)
                else:
                    emit_scores(i, [(16, 24), (24, 32)])
                    emit_rmax(i)
                    # last group's softmax first: its chain is the tail;
                    # group B-2's finishes in parallel
                    emit_softmax_attnv(i)
                    emit_softmax_attnv(i - 1)
            else:
                emit_rmax(i - 1)
                if i == B - 1:
                    emit_rmax(i)
                    emit_softmax_attnv(i)
                emit_softmax_attnv(i - 1)

    nc.compile()
    return nc


def _get_program(cfg):
    if cfg not in _prog_cache:
        _prog_cache[cfg] = _build_program(cfg)
    return _prog_cache[cfg]


def _width(L):
    # even-padded computed width; valid_len==0 means "uniform over all KV"
    if L <= 0:
        return KV
    L = min(L, KV)
    return min(KV, max(2, 2 * math.ceil(L / 2)))


def kernel(queries, keys, values, valid_lens, Wq, Wk, wv):
    global LAST_EXEC_NS
    queries = np.ascontiguousarray(np.asarray(queries), dtype=np.float32)
    keys = np.ascontiguousarray(np.asarray(keys), dtype=np.float32)
    values = np.ascontiguousarray(np.asarray(values), dtype=np.float32)
    Wq = np.ascontiguousarray(np.asarray(Wq), dtype=np.float32)
    Wk = np.ascontiguousarray(np.asarray(Wk), dtype=np.float32)
    wv = np.ascontiguousarray(np.asarray(wv), dtype=np.float32)
    vl = [int(x) for x in np.asarray(valid_lens)]

    W_b = [_width(L) for L in vl]
    # widest group first: its long tanh stream gives the DVE adds of every
    # later group enough runway; smallest group last for a short tail
    gorder = sorted(range(B), key=lambda b: (-W_b[b], b))
    Ws = tuple(W_b[b] for b in gorder)
    l0flags = tuple(vl[b] == 0 for b in gorder)
    Wmax = max(Ws)

    nc = _get_program((Ws, l0flags))

    kT = np.concatenate(
        [keys[gorder[i]][:Ws[i]].T for i in range(B)], axis=1)
    kT = np.ascontiguousarray(kT.astype(np.float16))     # [128, SW]
    Vm = np.ascontiguousarray(
        np.stack([values[gorder[i]] for i in range(B)]).astype(np.float16))
    # row 0: band indicator x per-group valid mask; row 1: outside-band
    # indicator x MASK_VAL (so non-band score rows exp to exactly 0)
    ind = np.zeros((2, B * P), np.float16)
    for i in range(B):
        ind[0, i * P + GROUP_ROWS * i: i * P + GROUP_ROWS * (i + 1)] = 1.0
        ind[1, i * P: (i + 1) * P] = 1.0
        ind[1, i * P + GROUP_ROWS * i: i * P + GROUP_ROWS * (i + 1)] = 0.0
    mask = np.zeros((2, B * Wmax), np.float16)
    mask[1, :] = MASK_VAL
    for i in range(B):
        L = vl[gorder[i]]
        if L > 0:
            mask[0, i * Wmax + min(L, Ws[i]): i * Wmax + Ws[i]] = MASK_VAL
    wvd = np.zeros((P, 2 * P - 1), np.float16)
    wvd[:, P - 1] = wv.astype(np.float16)
    bident = np.ascontiguousarray(
        np.tile(np.eye(GROUP_ROWS, dtype=np.float16), (B, 1)))

    blobq = np.zeros((P, 2 * P), np.float32)
    blobq[:, 0:P] = Wq
    shared = {"wk": np.ascontiguousarray(Wk.astype(np.float16)), "kT": kT, "V": Vm, "ind": ind,
              "mask": mask, "wvd": wvd, "bident": bident}
    in_maps = []
    for c in range(N_CORES):
        qT = np.concatenate(
            [queries[gorder[i], c * GROUP_ROWS:(c + 1) * GROUP_ROWS, :].T
             for i in range(B)], axis=1)
        bl = blobq.copy()
        bl[:, P:2 * P] = qT
        m = dict(shared)
        m["blobq"] = bl
        in_maps.append(m)

    if SIMULATE:
        from concourse.bass_interp import CoreSim
        outs = []
        for c in range(N_CORES):
            sim = CoreSim(nc, trace=False)
            for name, v in in_maps[c].items():
                sim.tensor(name)[:] = v
            sim.simulate(check_with_hw=False)
            outs.append(sim.tensor("out").copy())
    else:
        from concourse import bass_utils
        kw = {}
        if PROFILE:
            kw = {"trace": True}
        res = bass_utils.run_bass_kernel_spmd(nc, in_maps, list(range(N_CORES)),
                                              **kw)
        if PROFILE:
            LAST_EXEC_NS = res.exec_time_ns
            global LAST_RESULTS
            LAST_RESULTS = res
        outs = [res.results[c]["out"] for c in range(N_CORES)]

    out = np.zeros((B, Q, VS), np.float32)
    for c in range(N_CORES):
        for i in range(B):
            out[gorder[i], c * GROUP_ROWS:(c + 1) * GROUP_ROWS, :] = \
                outs[c][GROUP_ROWS * i: GROUP_ROWS * (i + 1), :]
    return out


# revision 37
# speedup vs baseline: 1.0315x; 1.0091x over previous
"""Additive attention (B=4, Q=256, KV=1024, H=128, VS=256) on 8 Trainium2 cores.

Sharding: each core processes 32 query rows of every batch (4 groups of 32
row-slots).  Per batch, only a KV prefix of width ~valid_len (padded to even)
is computed; masked columns beyond it contribute exactly 0 to the softmax, so
skipping them is exact.  No collectives.  The program is specialized per
valid_lens configuration at call time and cached.

Per-core dataflow (ACT tanh is the hard floor: 1 elem/cycle/lane,
dtype-independent, ScalarE-only):
  PE  : k/q projections in fp32 (accuracy-critical, pre-tanh)
  DVE : feats[h, kv] = fp16(kp16[h, kv] + qp[h, s])  (tensor_scalar add,
        fp16 4x mode; kp stored fp16, qp scalar fp32)
  ACT : tanh in place over 8-row batches (the throughput floor, ~50us)
  PE  : single-pass fp16 one-hot matmuls accumulate score rows into a
        per-group PSUM tile (wv window with the weight at column 127-s);
        each group's mask row is written first via one K=1 matmul
  DVE/ACT/PE: per-group masked softmax (reduce_max / exp-with-accum-out row
        sums -> fp32 probs), PE band transposes into one PSUM strip tile,
        one fp16 cast copy, attn @ V in fp16 32-column bands; group i's
        softmax+attnV hide under group i+1's tanh stream.  One reciprocal +
        scale at the very end.
Queue discipline: Wk arrives in its own first DMA so the k0 projection can
start immediately; kT loads are interleaved with group 0's adds; softmax of
group i is emitted after sub-batch 0 of group i+1's tanh so no engine queue
blocks on a cross-engine round trip; V-tile DMAs trail kT on the sync queue.
"""
import math
import os
import sys

import numpy as np

for _p in ("/opt/trn_rl_repo", "/root/.axon_site/_ro/trn_rl_repo"):
    if os.path.isdir(_p):
        if _p not in sys.path:
            sys.path.insert(0, _p)
        break

B, Q, KV, QS, KS, H, VS = 4, 256, 1024, 128, 128, 128, 256
P = 128
N_CORES = 8
GROUP_ROWS = 32          # rows per (core, batch)
SUB = 8                  # rows per tanh batch
MASK_VAL = -30000.0      # large-negative that still fits fp16

PROFILE = False          # set by test.py; enables NTFF tracing
LO_PASS = True           # kept for test.py compat (unused in v2)
LAST_RESULTS = None
SIMULATE = False         # set by test.py; run CoreSim instead of hardware
LAST_EXEC_NS = None

_prog_cache = {}


def _build_program(cfg):
    """cfg: (Ws, l0flags): per-group computed KV widths in processing order
    and per-group valid_len==0 flags.  Returns nc."""
    Ws, l0flags = cfg
    import contextlib

    import concourse.bacc as bacc
    import concourse.mybir as mybir
    import concourse.tile as tile
    from concourse.tile_rust import add_dep_helper

    f32 = mybir.dt.float32
    f16 = mybir.dt.float16
    W = list(Ws)
    Wmax = max(W)
    SW = sum(W)
    offs = [sum(W[:i]) for i in range(B)]          # kp_sb column offsets
    nstrips = [(w + P - 1) // P for w in W]
    NSB = GROUP_ROWS // SUB
    nc = bacc.Bacc("TRN2", target_bir_lowering=False, debug=False,
                   enable_asserts=True, num_devices=N_CORES)

    wk_d = nc.dram_tensor("wk", [P, P], f16, kind="ExternalInput").ap()
    blobq_d = nc.dram_tensor("blobq", [P, 2 * P], f32,
                             kind="ExternalInput").ap()
    kT_d = nc.dram_tensor("kT", [P, SW], f16, kind="ExternalInput").ap()
    V_d = nc.dram_tensor("V", [B, KV, VS], f16, kind="ExternalInput").ap()
    wvd_d = nc.dram_tensor("wvd", [P, 2 * P - 1], f16, kind="ExternalInput").ap()
    ind_d = nc.dram_tensor("ind", [2, B * P], f16, kind="ExternalInput").ap()
    mask_d = nc.dram_tensor("mask", [2, B * Wmax], f16, kind="ExternalInput").ap()
    bident_d = nc.dram_tensor("bident", [P, GROUP_ROWS], f16,
                              kind="ExternalInput").ap()
    out_d = nc.dram_tensor("out", [P, VS], f32, kind="ExternalOutput").ap()

    with tile.TileContext(nc) as tc, contextlib.ExitStack() as ctx:
        const = ctx.enter_context(tc.tile_pool(name="const", bufs=1))
        ktp = ctx.enter_context(tc.tile_pool(name="ktp", bufs=4))
        featsp = ctx.enter_context(tc.tile_pool(name="featsp", bufs=5))
        probsp = ctx.enter_context(tc.tile_pool(name="probsp", bufs=2))
        small = ctx.enter_context(tc.tile_pool(name="small", bufs=3))
        scp = ctx.enter_context(tc.tile_pool(name="scp", bufs=2, space="PSUM"))
        pmix = ctx.enter_context(tc.tile_pool(name="pmix", bufs=3, space="PSUM"))
        outp = ctx.enter_context(tc.tile_pool(name="outp", bufs=1, space="PSUM"))

        # ---- ACT table warm-up: load the exp/tanh spline set while the
        # first DMAs are still in flight ----
        warm = const.tile([1, 2], f16)
        nc.gpsimd.memset(warm[:], 0.0)
        nc.scalar.activation(warm[:], warm[:],
                             mybir.ActivationFunctionType.Tanh)

        # ---- constant loads: wk first (k0 projection gates the pipeline),
        # then kT0, then Wq + the first 32 qT columns (all group 0 needs),
        # then the rest; V tiles trail kT on the Sync queue; small leftovers
        # on the idle GpSimd issue queue ----
        wk_sb_t = const.tile([P, P], f16)
        nc.sync.dma_start(wk_sb_t[:], wk_d[:])
        wk_sb = wk_sb_t[:]
        blobq = const.tile([P, 2 * P], f32)
        wq_sb = blobq[:, 0:P]
        qt_sb = blobq[:, P:2 * P]
        wvd_t = const.tile([P, 2 * P - 1], f16)
        nc.gpsimd.dma_start(wvd_t[:], wvd_d[:])
        ind_sb = const.tile([2, B * P], f16)
        nc.gpsimd.dma_start(ind_sb[:], ind_d[:])
        mask_sb = const.tile([2, B * Wmax], f16)
        nc.gpsimd.dma_start(mask_sb[:], mask_d[:])
        bident = const.tile([P, GROUP_ROWS], f16)
        nc.gpsimd.dma_start(bident[:], bident_d[:])

        kp_sb = const.tile([P, SW], f16)
        out_ps = outp.tile([P, VS], f32, name="out_ps")
        out_sb = const.tile([P, VS], f32)
        rinv = small.tile([P, 1], f32, bufs=1, tag="rinv")
        vts = {}
        scores = [None] * B
        last_kp_copy = [None]

        kp_pss = {}

        def emit_load_mm(i, cs=512):
            """kT chunk DMAs + fp16 projections for group i (PE/sync
            side); chunked so each projection starts as soon as its
            columns land (group 0 uses finer chunks: it gates the head)."""
            w = W[i]
            kp_pss[i] = []
            for c0 in range(0, w, cs):
                n = min(cs, w - c0)
                kt_t = ktp.tile([P, 512], f16, tag="kt", name=f"kt_{i}_{c0}")
                nc.sync.dma_start(kt_t[:, :n], kT_d[:, offs[i] + c0:
                                                   offs[i] + c0 + n])
                kp_ps = pmix.tile([P, 512], f32, tag="mix",
                                  name=f"kp_ps_{i}_{c0}")
                nc.tensor.matmul(kp_ps[:, :n], wk_sb, kt_t[:, :n],
                                 start=True, stop=True)
                kp_pss[i].append((c0, n, kp_ps))

        def emit_load_cast(i):
            """fp16 kp copies for group i (DVE side, emitted late so the
            casts never block earlier adds)."""
            for c0, n, kp_ps in kp_pss[i]:
                last_kp_copy[0] = nc.vector.tensor_copy(
                    kp_sb[:, offs[i] + c0: offs[i] + c0 + n],
                    kp_ps[:, :n])

        def emit_mask(i):
            # K=2 rank-2 init: rows in the band get the valid-len mask, rows
            # outside it get MASK_VAL so they exp to exactly 0 later (the
            # probs->pt reduction matmul sums over all four bands).
            w = W[i]
            sc = scp.tile([P, w], f32, tag="sc", name=f"scores_{i}")
            scores[i] = sc
            for c0 in range(0, w, 512):
                c1 = min(c0 + 512, w)
                nc.tensor.matmul(
                    sc[:, c0:c1],
                    ind_sb[0:2, i * P:(i + 1) * P],
                    mask_sb[0:2, i * Wmax + c0: i * Wmax + c1],
                    start=True, stop=l0flags[i] and c1 == w,
                    skip_group_check=True)

        def emit_scores(i, ranges):
            """adds + tanh + one-hot score matmuls for group i over the
            given (row0, row1) ranges."""
            w = W[i]
            sc = scores[i]
            for r0, r1 in ranges:
                nr = r1 - r0
                feats = featsp.tile([P, nr * w], f16, tag="feats",
                                    name=f"feats_{i}_{r0}")
                for j in range(nr):
                    s = GROUP_ROWS * i + r0 + j
                    nc.vector.tensor_scalar_add(
                        feats[:, j * w:(j + 1) * w],
                        kp_sb[:, offs[i]: offs[i] + w],
                        qp_sb[:, s: s + 1])
                nc.scalar.activation(feats[:], feats[:],
                                     mybir.ActivationFunctionType.Tanh)
                for j in range(nr):
                    s = GROUP_ROWS * i + r0 + j
                    last_row = r0 + j == GROUP_ROWS - 1
                    for c0 in range(0, w, 512):
                        c1 = min(c0 + 512, w)
                        nc.tensor.matmul(
                            sc[:, c0:c1],
                            wvd_t[:, P - 1 - s: 2 * P - 1 - s],
                            feats[:, j * w + c0: j * w + c1],
                            start=False,
                            stop=last_row and c1 == w,
                            skip_group_check=True)

        def emit_vdma(i):
            for c in range(nstrips[i]):
                cw = min(P, W[i] - c * P)
                vts[(i, c)] = const.tile([P, VS], f16, name=f"v_{i}_{c}")
                vdma = nc.sync.dma_start(vts[(i, c)][:cw, :],
                                         V_d[i, c * P: c * P + cw, :])
                if last_kp_copy[0] is not None:
                    add_dep_helper(vdma.ins, last_kp_copy[0].ins,
                                   reason="V after kp: kT wins head HBM bw")

        nrms = [None] * B

        def emit_rmax(i):
            # per-group -max; 0 outside the band so those rows (scores
            # MASK_VAL) exp to exactly 0.  Split from the rest of the
            # softmax so the last group's reduce_max is not queued on DVE
            # behind the previous group's exp-gated ops.
            sc = scores[i]
            band = slice(GROUP_ROWS * i, GROUP_ROWS * (i + 1))
            nrm = small.tile([P, 1], f32, bufs=2, tag="nrm",
                             name=f"nrm_{i}")
            nrms[i] = nrm
            nc.vector.memset(nrm[:], 0.0)
            nc.vector.reduce_max(nrm[band, :], sc[band, :],
                                 axis=mybir.AxisListType.X, negate=True)

        def emit_softmax_attnv(i):
            w = W[i]
            n = nstrips[i]
            sc = scores[i]
            nrm = nrms[i]
            band = slice(GROUP_ROWS * i, GROUP_ROWS * (i + 1))
            wpad = n * P
            probs = probsp.tile([P, wpad], f16, tag="probs",
                                name=f"probs_{i}")
            rs = small.tile([P, 1], f32, bufs=4, tag="rs", name=f"rs_{i}")
            if wpad > w:
                nc.vector.memset(probs[:, w:], 0.0)
            nc.scalar.activation(probs[:, :w], sc[:],
                                 mybir.ActivationFunctionType.Exp,
                                 bias=nrm[:, 0:1], scale=1.0,
                                 accum_out=rs[:, 0:1])
            nc.vector.reciprocal(rinv[band, :], rs[band, :])
            # "transpose" probs via probs.T @ band-identity: full-height
            # stationary (base partition 0 -- quadrant-3-safe); non-band
            # rows are exactly 0 so the cross-band sum picks out the band
            pt_ps = pmix.tile([P, GROUP_ROWS * n], f32, tag="mix",
                              name=f"pt_ps_{i}")
            for c in range(n):
                nc.tensor.matmul(pt_ps[:, GROUP_ROWS * c:
                                       GROUP_ROWS * (c + 1)],
                                 probs[:, c * P:(c + 1) * P],
                                 bident[:, :],
                                 start=True, stop=True,
                                 skip_group_check=True)
            pt_sb = small.tile([P, GROUP_ROWS * n], f16, tag="pt",
                               name=f"pt_sb_{i}")
            nc.vector.tensor_copy(pt_sb[:], pt_ps[:])
            for c in range(n):
                cw = min(P, w - c * P)
                nc.tensor.matmul(
                    out_ps[band, :],
                    pt_sb[:cw, GROUP_ROWS * c: GROUP_ROWS * (c + 1)],
                    vts[(i, c)][:cw, :],
                    start=(c == 0), stop=(c == n - 1),
                    tile_position=(0, GROUP_ROWS * i),
                    skip_group_check=True)
            # scale + ship this band now: only the last group's output DMA
            # lands in the tail
            nc.vector.tensor_scalar_mul(out_sb[band, :], out_ps[band, :],
                                        rinv[band, 0:1])
            nc.sync.dma_start(out_d[GROUP_ROWS * i: GROUP_ROWS * (i + 1), :],
                              out_sb[band, :])

        # ---- head: kT0 + group-0 q columns first on the sync queue (the
        # GpSimd queue's SWDGE descriptor generation is ~650ns per DMA,
        # too slow for the head-critical q data) ----
        emit_load_mm(0, cs=256)
        nc.sync.dma_start(wq_sb, blobq_d[:, 0:P])
        nc.sync.dma_start(qt_sb[:, 0:GROUP_ROWS],
                          blobq_d[:, P:P + GROUP_ROWS])
        nc.sync.dma_start(qt_sb[:, GROUP_ROWS:],
                          blobq_d[:, P + GROUP_ROWS:2 * P])
        qp_ps = pmix.tile([P, P], f32, tag="mix", name="qp_ps")
        nc.tensor.matmul(qp_ps[:, 0:GROUP_ROWS], wq_sb,
                         qt_sb[:, 0:GROUP_ROWS], start=True, stop=True,
                         skip_group_check=True)
        qp_sb = const.tile([P, P], f32)
        nc.vector.tensor_copy(qp_sb[:, 0:GROUP_ROWS], qp_ps[:, 0:GROUP_ROWS])
        nc.tensor.matmul(qp_ps[:, GROUP_ROWS:], wq_sb,
                         qt_sb[:, GROUP_ROWS:], start=True, stop=True,
                         skip_group_check=True)
        nc.vector.tensor_copy(qp_sb[:, GROUP_ROWS:], qp_ps[:, GROUP_ROWS:])
        emit_load_cast(0)
        emit_mask(0)
        if not l0flags[0]:
            emit_scores(0, [(0, 4), (4, 8)])
            emit_load_mm(1)
            emit_scores(0, [(8, 16)])
            emit_load_mm(2)
            emit_load_mm(3)
            emit_scores(0, [(16, 32)])
        else:
            emit_load_mm(1)
            emit_load_mm(2)
            emit_load_mm(3)
        emit_load_cast(1)
        emit_vdma(0)

        # ---- main loop: group i's first tanh batch precedes group i-1's
        # softmax; the last group's reduce_max precedes the exp-gated DVE
        # work of groups B-2/B-1 so the tail chain starts immediately ----
        for i in range(1, B):
            emit_mask(i)
            emit_vdma(i)
            if i + 1 < B:
                emit_load_cast(i + 1)
            if not l0flags[i]:
                emit_scores(i, [(0, 16)])
                emit_rmax(i - 1)
                if i < B - 1:
                    emit_scores(i, [(16, 32)])
                    emit_softmax_attnv(i - 1)
                else:
                    emit_scores(i, [(16, 28), (28, 32)])
                    emit_rmax(i)
                    emit_softmax_attnv(i - 1)
                    emit_softmax_attnv(i)
            else:
                emit_rmax(i - 1)
                emit_softmax_attnv(i - 1)
                if i == B - 1:
                    emit_rmax(i)
                    emit_softmax_attnv(i)

    nc.compile()
    return nc


def _get_program(cfg):
    if cfg not in _prog_cache:
        _prog_cache[cfg] = _build_program(cfg)
    return _prog_cache[cfg]


def _width(L):
    # even-padded computed width; valid_len==0 means "uniform over all KV"
    if L <= 0:
        return KV
    L = min(L, KV)
    return min(KV, max(2, 2 * math.ceil(L / 2)))


def kernel(queries, keys, values, valid_lens, Wq, Wk, wv):
    global LAST_EXEC_NS
    queries = np.ascontiguousarray(np.asarray(queries), dtype=np.float32)
    keys = np.ascontiguousarray(np.asarray(keys), dtype=np.float32)
    values = np.ascontiguousarray(np.asarray(values), dtype=np.float32)
    Wq = np.ascontiguousarray(np.asarray(Wq), dtype=np.float32)
    Wk = np.ascontiguousarray(np.asarray(Wk), dtype=np.float32)
    wv = np.ascontiguousarray(np.asarray(wv), dtype=np.float32)
    vl = [int(x) for x in np.asarray(valid_lens)]

    W_b = [_width(L) for L in vl]
    # widest group first: its long tanh stream gives the DVE adds of every
    # later group enough runway; smallest group last for a short tail
    gorder = sorted(range(B), key=lambda b: (-W_b[b], b))
    Ws = tuple(W_b[b] for b in gorder)
    l0flags = tuple(vl[b] == 0 for b in gorder)
    Wmax = max(Ws)

    nc = _get_program((Ws, l0flags))

    kT = np.concatenate(
        [keys[gorder[i]][:Ws[i]].T for i in range(B)], axis=1)
    kT = np.ascontiguousarray(kT.astype(np.float16))     # [128, SW]
    Vm = np.ascontiguousarray(
        np.stack([values[gorder[i]] for i in range(B)]).astype(np.float16))
    # row 0: band indicator x per-group valid mask; row 1: outside-band
    # indicator x MASK_VAL (so non-band score rows exp to exactly 0)
    ind = np.zeros((2, B * P), np.float16)
    for i in range(B):
        ind[0, i * P + GROUP_ROWS * i: i * P + GROUP_ROWS * (i + 1)] = 1.0
        ind[1, i * P: (i + 1) * P] = 1.0
        ind[1, i * P + GROUP_ROWS * i: i * P + GROUP_ROWS * (i + 1)] = 0.0
    mask = np.zeros((2, B * Wmax), np.float16)
    mask[1, :] = MASK_VAL
    for i in range(B):
        L = vl[gorder[i]]
        if L > 0:
            mask[0, i * Wmax + min(L, Ws[i]): i * Wmax + Ws[i]] = MASK_VAL
    wvd = np.zeros((P, 2 * P - 1), np.float16)
    wvd[:, P - 1] = wv.astype(np.float16)
    bident = np.ascontiguousarray(
        np.tile(np.eye(GROUP_ROWS, dtype=np.float16), (B, 1)))

    blobq = np.zeros((P, 2 * P), np.float32)
    blobq[:, 0:P] = Wq
    shared = {"wk": np.ascontiguousarray(Wk.astype(np.float16)), "kT": kT, "V": Vm, "ind": ind,
              "mask": mask, "wvd": wvd, "bident": bident}
    in_maps = []
    for c in range(N_CORES):
        qT = np.concatenate(
            [queries[gorder[i], c * GROUP_ROWS:(c + 1) * GROUP_ROWS, :].T
             for i in range(B)], axis=1)
        bl = blobq.copy()
        bl[:, P:2 * P] = qT
        m = dict(shared)
        m["blobq"] = bl
        in_maps.append(m)

    if SIMULATE:
        from concourse.bass_interp import CoreSim
        outs = []
        for c in range(N_CORES):
            sim = CoreSim(nc, trace=False)
            for name, v in in_maps[c].items():
                sim.tensor(name)[:] = v
            sim.simulate(check_with_hw=False)
            outs.append(sim.tensor("out").copy())
    else:
        from concourse import bass_utils
        kw = {}
        if PROFILE:
            kw = {"trace": True}
        res = bass_utils.run_bass_kernel_spmd(nc, in_maps, list(range(N_CORES)),
                                              **kw)
        if PROFILE:
            LAST_EXEC_NS = res.exec_time_ns
            global LAST_RESULTS
            LAST_RESULTS = res
        outs = [res.results[c]["out"] for c in range(N_CORES)]

    out = np.zeros((B, Q, VS), np.float32)
    for c in range(N_CORES):
        for i in range(B):
            out[gorder[i], c * GROUP_ROWS:(c + 1) * GROUP_ROWS, :] = \
                outs[c][GROUP_ROWS * i: GROUP_ROWS * (i + 1), :]
    return out


# revision 39
# speedup vs baseline: 1.0402x; 1.0084x over previous
"""Additive attention (B=4, Q=256, KV=1024, H=128, VS=256) on 8 Trainium2 cores.

Sharding: each core processes 32 query rows of every batch (4 groups of 32
row-slots).  Per batch, only a KV prefix of width ~valid_len (padded to even)
is computed; masked columns beyond it contribute exactly 0 to the softmax, so
skipping them is exact.  No collectives.  The program is specialized per
valid_lens configuration at call time and cached.

Per-core dataflow (ACT tanh is the hard floor: 1 elem/cycle/lane,
dtype-independent, ScalarE-only):
  PE  : k/q projections in fp32 (accuracy-critical, pre-tanh)
  DVE : feats[h, kv] = fp16(kp16[h, kv] + qp[h, s])  (tensor_scalar add,
        fp16 4x mode; kp stored fp16, qp scalar fp32)
  ACT : tanh in place over 8-row batches (the throughput floor, ~50us)
  PE  : single-pass fp16 one-hot matmuls accumulate score rows into a
        per-group PSUM tile (wv window with the weight at column 127-s);
        each group's mask row is written first via one K=1 matmul
  DVE/ACT/PE: per-group masked softmax (reduce_max / exp-with-accum-out row
        sums -> fp32 probs), PE band transposes into one PSUM strip tile,
        one fp16 cast copy, attn @ V in fp16 32-column bands; group i's
        softmax+attnV hide under group i+1's tanh stream.  One reciprocal +
        scale at the very end.
Queue discipline: Wk arrives in its own first DMA so the k0 projection can
start immediately; kT loads are interleaved with group 0's adds; softmax of
group i is emitted after sub-batch 0 of group i+1's tanh so no engine queue
blocks on a cross-engine round trip; V-tile DMAs trail kT on the sync queue.
"""
import math
import os
import sys

import numpy as np

for _p in ("/opt/trn_rl_repo", "/root/.axon_site/_ro/trn_rl_repo"):
    if os.path.isdir(_p):
        if _p not in sys.path:
            sys.path.insert(0, _p)
        break

B, Q, KV, QS, KS, H, VS = 4, 256, 1024, 128, 128, 128, 256
P = 128
N_CORES = 8
GROUP_ROWS = 32          # rows per (core, batch)
SUB = 8                  # rows per tanh batch
MASK_VAL = -30000.0      # large-negative that still fits fp16

PROFILE = False          # set by test.py; enables NTFF tracing
LO_PASS = True           # kept for test.py compat (unused in v2)
LAST_RESULTS = None
SIMULATE = False         # set by test.py; run CoreSim instead of hardware
LAST_EXEC_NS = None

_prog_cache = {}


def _build_program(cfg):
    """cfg: (Ws, l0flags): per-group computed KV widths in processing order
    and per-group valid_len==0 flags.  Returns nc."""
    Ws, l0flags = cfg
    import contextlib

    import concourse.bacc as bacc
    import concourse.mybir as mybir
    import concourse.tile as tile
    from concourse.tile_rust import add_dep_helper

    f32 = mybir.dt.float32
    f16 = mybir.dt.float16
    W = list(Ws)
    Wmax = max(W)
    SW = sum(W)
    offs = [sum(W[:i]) for i in range(B)]          # kp_sb column offsets
    nstrips = [(w + P - 1) // P for w in W]
    NSB = GROUP_ROWS // SUB
    nc = bacc.Bacc("TRN2", target_bir_lowering=False, debug=False,
                   enable_asserts=True, num_devices=N_CORES)

    wk_d = nc.dram_tensor("wk", [P, P], f16, kind="ExternalInput").ap()
    blobq_d = nc.dram_tensor("blobq", [P, 2 * P], f32,
                             kind="ExternalInput").ap()
    kT_d = nc.dram_tensor("kT", [P, SW], f16, kind="ExternalInput").ap()
    V_d = nc.dram_tensor("V", [B, KV, VS], f16, kind="ExternalInput").ap()
    wvd_d = nc.dram_tensor("wvd", [P, 2 * P - 1], f16, kind="ExternalInput").ap()
    ind_d = nc.dram_tensor("ind", [2, B * P], f16, kind="ExternalInput").ap()
    mask_d = nc.dram_tensor("mask", [2, B * Wmax], f16, kind="ExternalInput").ap()
    bident_d = nc.dram_tensor("bident", [P, GROUP_ROWS], f16,
                              kind="ExternalInput").ap()
    out_d = nc.dram_tensor("out", [P, VS], f32, kind="ExternalOutput").ap()

    with tile.TileContext(nc) as tc, contextlib.ExitStack() as ctx:
        const = ctx.enter_context(tc.tile_pool(name="const", bufs=1))
        ktp = ctx.enter_context(tc.tile_pool(name="ktp", bufs=4))
        featsp = ctx.enter_context(tc.tile_pool(name="featsp", bufs=5))
        probsp = ctx.enter_context(tc.tile_pool(name="probsp", bufs=2))
        small = ctx.enter_context(tc.tile_pool(name="small", bufs=3))
        scp = ctx.enter_context(tc.tile_pool(name="scp", bufs=2, space="PSUM"))
        pmix = ctx.enter_context(tc.tile_pool(name="pmix", bufs=3, space="PSUM"))
        outp = ctx.enter_context(tc.tile_pool(name="outp", bufs=1, space="PSUM"))

        # ---- ACT table warm-up: load the exp/tanh spline set while the
        # first DMAs are still in flight ----
        warm = const.tile([1, 2], f16)
        nc.gpsimd.memset(warm[:], 0.0)
        nc.scalar.activation(warm[:], warm[:],
                             mybir.ActivationFunctionType.Tanh)

        # ---- constant loads: wk first (k0 projection gates the pipeline),
        # then kT0, then Wq + the first 32 qT columns (all group 0 needs),
        # then the rest; V tiles trail kT on the Sync queue; small leftovers
        # on the idle GpSimd issue queue ----
        wk_sb_t = const.tile([P, P], f16)
        nc.sync.dma_start(wk_sb_t[:], wk_d[:])
        wk_sb = wk_sb_t[:]
        blobq = const.tile([P, 2 * P], f32)
        wq_sb = blobq[:, 0:P]
        qt_sb = blobq[:, P:2 * P]
        wvd_t = const.tile([P, 2 * P - 1], f16)
        nc.gpsimd.dma_start(wvd_t[:], wvd_d[:])
        ind_sb = const.tile([2, B * P], f16)
        nc.gpsimd.dma_start(ind_sb[:], ind_d[:])
        mask_sb = const.tile([2, B * Wmax], f16)
        nc.gpsimd.dma_start(mask_sb[:], mask_d[:])
        bident = const.tile([P, GROUP_ROWS], f16)
        nc.gpsimd.dma_start(bident[:], bident_d[:])

        kp_sb = const.tile([P, SW], f16)
        out_ps = outp.tile([P, VS], f32, name="out_ps")
        out_sb = const.tile([P, VS], f32)
        rinv = small.tile([P, 1], f32, bufs=1, tag="rinv")
        vts = {}
        scores = [None] * B
        last_kp_copy = [None]

        kp_pss = {}

        def emit_load_mm(i, cs=512):
            """kT chunk DMAs + fp16 projections for group i (PE/sync
            side); chunked so each projection starts as soon as its
            columns land (group 0 uses finer chunks: it gates the head)."""
            w = W[i]
            kp_pss[i] = []
            for c0 in range(0, w, cs):
                n = min(cs, w - c0)
                kt_t = ktp.tile([P, 512], f16, tag="kt", name=f"kt_{i}_{c0}")
                nc.sync.dma_start(kt_t[:, :n], kT_d[:, offs[i] + c0:
                                                   offs[i] + c0 + n])
                kp_ps = pmix.tile([P, 512], f32, tag="mix",
                                  name=f"kp_ps_{i}_{c0}")
                nc.tensor.matmul(kp_ps[:, :n], wk_sb, kt_t[:, :n],
                                 start=True, stop=True)
                kp_pss[i].append((c0, n, kp_ps))

        def emit_load_cast(i):
            """fp16 kp copies for group i (DVE side, emitted late so the
            casts never block earlier adds)."""
            for c0, n, kp_ps in kp_pss[i]:
                last_kp_copy[0] = nc.vector.tensor_copy(
                    kp_sb[:, offs[i] + c0: offs[i] + c0 + n],
                    kp_ps[:, :n])

        def emit_mask(i):
            # K=2 rank-2 init: rows in the band get the valid-len mask, rows
            # outside it get MASK_VAL so they exp to exactly 0 later (the
            # probs->pt reduction matmul sums over all four bands).
            w = W[i]
            sc = scp.tile([P, w], f32, tag="sc", name=f"scores_{i}")
            scores[i] = sc
            for c0 in range(0, w, 512):
                c1 = min(c0 + 512, w)
                nc.tensor.matmul(
                    sc[:, c0:c1],
                    ind_sb[0:2, i * P:(i + 1) * P],
                    mask_sb[0:2, i * Wmax + c0: i * Wmax + c1],
                    start=True, stop=l0flags[i] and c1 == w,
                    skip_group_check=True)

        def emit_scores(i, ranges):
            """adds + tanh + one-hot score matmuls for group i over the
            given (row0, row1) ranges."""
            w = W[i]
            sc = scores[i]
            for r0, r1 in ranges:
                nr = r1 - r0
                feats = featsp.tile([P, nr * w], f16, tag="feats",
                                    name=f"feats_{i}_{r0}")
                for j in range(nr):
                    s = GROUP_ROWS * i + r0 + j
                    nc.vector.tensor_scalar_add(
                        feats[:, j * w:(j + 1) * w],
                        kp_sb[:, offs[i]: offs[i] + w],
                        qp_sb[:, s: s + 1])
                nc.scalar.activation(feats[:], feats[:],
                                     mybir.ActivationFunctionType.Tanh)
                for j in range(nr):
                    s = GROUP_ROWS * i + r0 + j
                    last_row = r0 + j == GROUP_ROWS - 1
                    for c0 in range(0, w, 512):
                        c1 = min(c0 + 512, w)
                        nc.tensor.matmul(
                            sc[:, c0:c1],
                            wvd_t[:, P - 1 - s: 2 * P - 1 - s],
                            feats[:, j * w + c0: j * w + c1],
                            start=False,
                            stop=last_row and c1 == w,
                            skip_group_check=True)

        def emit_vdma(i):
            for c in range(nstrips[i]):
                cw = min(P, W[i] - c * P)
                vts[(i, c)] = const.tile([P, VS], f16, name=f"v_{i}_{c}")
                vdma = nc.sync.dma_start(vts[(i, c)][:cw, :],
                                         V_d[i, c * P: c * P + cw, :])
                if last_kp_copy[0] is not None:
                    add_dep_helper(vdma.ins, last_kp_copy[0].ins,
                                   reason="V after kp: kT wins head HBM bw")

        nrms = [None] * B

        def emit_rmax(i):
            # per-group -max; 0 outside the band so those rows (scores
            # MASK_VAL) exp to exactly 0.  Split from the rest of the
            # softmax so the last group's reduce_max is not queued on DVE
            # behind the previous group's exp-gated ops.
            sc = scores[i]
            band = slice(GROUP_ROWS * i, GROUP_ROWS * (i + 1))
            nrm = small.tile([P, 1], f32, bufs=2, tag="nrm",
                             name=f"nrm_{i}")
            nrms[i] = nrm
            nc.vector.memset(nrm[:], 0.0)
            nc.vector.reduce_max(nrm[band, :], sc[band, :],
                                 axis=mybir.AxisListType.X, negate=True)

        def emit_softmax_attnv(i):
            w = W[i]
            n = nstrips[i]
            sc = scores[i]
            nrm = nrms[i]
            band = slice(GROUP_ROWS * i, GROUP_ROWS * (i + 1))
            wpad = n * P
            probs = probsp.tile([P, wpad], f16, tag="probs",
                                name=f"probs_{i}")
            rs = small.tile([P, 1], f32, bufs=4, tag="rs", name=f"rs_{i}")
            if wpad > w:
                nc.vector.memset(probs[:, w:], 0.0)
            nc.scalar.activation(probs[:, :w], sc[:],
                                 mybir.ActivationFunctionType.Exp,
                                 bias=nrm[:, 0:1], scale=1.0,
                                 accum_out=rs[:, 0:1])
            nc.vector.reciprocal(rinv[band, :], rs[band, :])
            # "transpose" probs via probs.T @ band-identity: full-height
            # stationary (base partition 0 -- quadrant-3-safe); non-band
            # rows are exactly 0 so the cross-band sum picks out the band
            pt_ps = pmix.tile([P, GROUP_ROWS * n], f32, tag="mix",
                              name=f"pt_ps_{i}")
            for c in range(n):
                nc.tensor.matmul(pt_ps[:, GROUP_ROWS * c:
                                       GROUP_ROWS * (c + 1)],
                                 probs[:, c * P:(c + 1) * P],
                                 bident[:, :],
                                 start=True, stop=True,
                                 skip_group_check=True)
            pt_sb = small.tile([P, GROUP_ROWS * n], f16, tag="pt",
                               name=f"pt_sb_{i}")
            nc.vector.tensor_copy(pt_sb[:], pt_ps[:])
            for c in range(n):
                cw = min(P, w - c * P)
                nc.tensor.matmul(
                    out_ps[band, :],
                    pt_sb[:cw, GROUP_ROWS * c: GROUP_ROWS * (c + 1)],
                    vts[(i, c)][:cw, :],
                    start=(c == 0), stop=(c == n - 1),
                    tile_position=(0, GROUP_ROWS * i),
                    skip_group_check=True)
            # scale + ship this band now: only the last group's output DMA
            # lands in the tail
            nc.vector.tensor_scalar_mul(out_sb[band, :], out_ps[band, :],
                                        rinv[band, 0:1])
            nc.sync.dma_start(out_d[GROUP_ROWS * i: GROUP_ROWS * (i + 1), :],
                              out_sb[band, :])

        # ---- head: q-side data for group 0 (Wq + its 32 qT columns)
        # before kT0 on the sync queue; the rest of qp is projected after
        # group 0's first adds are underway ----
        nc.sync.dma_start(wq_sb, blobq_d[:, 0:P])
        nc.sync.dma_start(qt_sb[:, 0:GROUP_ROWS],
                          blobq_d[:, P:P + GROUP_ROWS])
        qp_ps = pmix.tile([P, P], f32, tag="mix", name="qp_ps")
        nc.tensor.matmul(qp_ps[:, 0:GROUP_ROWS], wq_sb,
                         qt_sb[:, 0:GROUP_ROWS], start=True, stop=True,
                         skip_group_check=True)
        qp_sb = const.tile([P, P], f32)
        nc.vector.tensor_copy(qp_sb[:, 0:GROUP_ROWS], qp_ps[:, 0:GROUP_ROWS])
        emit_load_mm(0, cs=384)
        nc.sync.dma_start(qt_sb[:, GROUP_ROWS:],
                          blobq_d[:, P + GROUP_ROWS:2 * P])
        emit_load_cast(0)
        emit_mask(0)
        if not l0flags[0]:
            emit_scores(0, [(0, 2), (2, 6), (6, 8)])
            nc.tensor.matmul(qp_ps[:, GROUP_ROWS:], wq_sb,
                             qt_sb[:, GROUP_ROWS:], start=True, stop=True,
                             skip_group_check=True)
            nc.vector.tensor_copy(qp_sb[:, GROUP_ROWS:],
                                  qp_ps[:, GROUP_ROWS:])
            emit_load_mm(1)
            emit_scores(0, [(8, 16)])
            emit_load_mm(2)
            emit_load_mm(3)
            emit_scores(0, [(16, 32)])
        else:
            nc.tensor.matmul(qp_ps[:, GROUP_ROWS:], wq_sb,
                             qt_sb[:, GROUP_ROWS:], start=True, stop=True,
                             skip_group_check=True)
            nc.vector.tensor_copy(qp_sb[:, GROUP_ROWS:],
                                  qp_ps[:, GROUP_ROWS:])
            emit_load_mm(1)
            emit_load_mm(2)
            emit_load_mm(3)
        emit_load_cast(1)
        emit_vdma(0)

        # ---- main loop: group i's first tanh batch precedes group i-1's
        # softmax; the last group's reduce_max precedes the exp-gated DVE
        # work of groups B-2/B-1 so the tail chain starts immediately ----
        for i in range(1, B):
            emit_mask(i)
            emit_vdma(i)
            if i + 1 < B:
                emit_load_cast(i + 1)
            if not l0flags[i]:
                emit_scores(i, [(0, 16)])
                emit_rmax(i - 1)
                if i < B - 1:
                    emit_scores(i, [(16, 32)])
                    emit_softmax_attnv(i - 1)
                else:
                    emit_scores(i, [(16, 28), (28, 32)])
                    emit_rmax(i)
                    emit_softmax_attnv(i - 1)
                    emit_softmax_attnv(i)
            else:
                emit_rmax(i - 1)
                emit_softmax_attnv(i - 1)
                if i == B - 1:
                    emit_rmax(i)
                    emit_softmax_attnv(i)

    nc.compile()
    return nc


def _get_program(cfg):
    if cfg not in _prog_cache:
        _prog_cache[cfg] = _build_program(cfg)
    return _prog_cache[cfg]


def _width(L):
    # even-padded computed width; valid_len==0 means "uniform over all KV"
    if L <= 0:
        return KV
    L = min(L, KV)
    return min(KV, max(2, 2 * math.ceil(L / 2)))


def kernel(queries, keys, values, valid_lens, Wq, Wk, wv):
    global LAST_EXEC_NS
    queries = np.ascontiguousarray(np.asarray(queries), dtype=np.float32)
    keys = np.ascontiguousarray(np.asarray(keys), dtype=np.float32)
    values = np.ascontiguousarray(np.asarray(values), dtype=np.float32)
    Wq = np.ascontiguousarray(np.asarray(Wq), dtype=np.float32)
    Wk = np.ascontiguousarray(np.asarray(Wk), dtype=np.float32)
    wv = np.ascontiguousarray(np.asarray(wv), dtype=np.float32)
    vl = [int(x) for x in np.asarray(valid_lens)]

    W_b = [_width(L) for L in vl]
    # widest group first: its long tanh stream gives the DVE adds of every
    # later group enough runway; smallest group last for a short tail
    gorder = sorted(range(B), key=lambda b: (-W_b[b], b))
    Ws = tuple(W_b[b] for b in gorder)
    l0flags = tuple(vl[b] == 0 for b in gorder)
    Wmax = max(Ws)

    nc = _get_program((Ws, l0flags))

    kT = np.concatenate(
        [keys[gorder[i]][:Ws[i]].T for i in range(B)], axis=1)
    kT = np.ascontiguousarray(kT.astype(np.float16))     # [128, SW]
    Vm = np.ascontiguousarray(
        np.stack([values[gorder[i]] for i in range(B)]).astype(np.float16))
    # row 0: band indicator x per-group valid mask; row 1: outside-band
    # indicator x MASK_VAL (so non-band score rows exp to exactly 0)
    ind = np.zeros((2, B * P), np.float16)
    for i in range(B):
        ind[0, i * P + GROUP_ROWS * i: i * P + GROUP_ROWS * (i + 1)] = 1.0
        ind[1, i * P: (i + 1) * P] = 1.0
        ind[1, i * P + GROUP_ROWS * i: i * P + GROUP_ROWS * (i + 1)] = 0.0
    mask = np.zeros((2, B * Wmax), np.float16)
    mask[1, :] = MASK_VAL
    for i in range(B):
        L = vl[gorder[i]]
        if L > 0:
            mask[0, i * Wmax + min(L, Ws[i]): i * Wmax + Ws[i]] = MASK_VAL
    wvd = np.zeros((P, 2 * P - 1), np.float16)
    wvd[:, P - 1] = wv.astype(np.float16)
    bident = np.ascontiguousarray(
        np.tile(np.eye(GROUP_ROWS, dtype=np.float16), (B, 1)))

    blobq = np.zeros((P, 2 * P), np.float32)
    blobq[:, 0:P] = Wq
    shared = {"wk": np.ascontiguousarray(Wk.astype(np.float16)), "kT": kT, "V": Vm, "ind": ind,
              "mask": mask, "wvd": wvd, "bident": bident}
    in_maps = []
    for c in range(N_CORES):
        qT = np.concatenate(
            [queries[gorder[i], c * GROUP_ROWS:(c + 1) * GROUP_ROWS, :].T
             for i in range(B)], axis=1)
        bl = blobq.copy()
        bl[:, P:2 * P] = qT
        m = dict(shared)
        m["blobq"] = bl
        in_maps.append(m)

    if SIMULATE:
        from concourse.bass_interp import CoreSim
        outs = []
        for c in range(N_CORES):
            sim = CoreSim(nc, trace=False)
            for name, v in in_maps[c].items():
                sim.tensor(name)[:] = v
            sim.simulate(check_with_hw=False)
            outs.append(sim.tensor("out").copy())
    else:
        from concourse import bass_utils
        kw = {}
        if PROFILE:
            kw = {"trace": True}
        res = bass_utils.run_bass_kernel_spmd(nc, in_maps, list(range(N_CORES)),
                                              **kw)
        if PROFILE:
            LAST_EXEC_NS = res.exec_time_ns
            global LAST_RESULTS
            LAST_RESULTS = res
        outs = [res.results[c]["out"] for c in range(N_CORES)]

    out = np.zeros((B, Q, VS), np.float32)
    for c in range(N_CORES):
        for i in range(B):
            out[gorder[i], c * GROUP_ROWS:(c + 1) * GROUP_ROWS, :] = \
                outs[c][GROUP_ROWS * i: GROUP_ROWS * (i + 1), :]
    return out
